# revision 26
# baseline (speedup 1.0000x reference)
"""Causal single-head attention (B=8, S=2048, E=1024, H=64) on 8 TRN2 cores.

Data-parallel over batch: core b handles batch element b end-to-end.

Per-core algorithm (all layouts chosen so every matmul contraction sits on
the SBUF partition dim):
  inputs (host-prepped): xT [E,S] (x transposed), W = [8*Wq | Wv | Wk]
  stacked with biases as an extra rank-1 contraction chunk (ones row x
  bias row), prepacked to the SBUF chunk layout.
  1) Projection, W-stationary: per 512-col s-block, accumulate over 8
     E-chunks plus the bias chunk: psum_qv[128,512] (rows 0:64 = Q^T,
     rows 64:128 = V^T), psum_k[64,512] = K^T.  PSUM -> SBUF moves are
     casting copies on DVE (q,k) and ACT (v, to bf16).
  2) V^T 128-col blocks PE-transposed (bf16) back to natural V [s,64];
     ones column -> V_aug so row 64 of the O accumulator collects Z.
  3) Pass 1 (per q-tile): scores over the previous + diagonal 128-blocks
     (>=129 causal keys per row), diagonal masked, row-max -> m.  Any
     per-row shift within exp range of the true causal max works (it
     cancels through the final normalization).
  4) -(m+10) -> row 64 of Q_aug (PE transpose + negate + strided DMA); row
     64 of K_aug = 1.0, so pass-2 scores come out pre-shifted: k.q - m_q.
  5) Pass 2 per k-chunk j: scoresT[k,q] blocks, mask, exp (ACT) -> wei^T;
     O^T[h',q] += V_aug.T @ wei^T accumulated over j in PSUM [65,2048].
     The next block's projection/pass-1 matmuls are interleaved between
     score groups as PE filler so the exp latency never starves the PE.
  6) PE-transpose O^T [65,128] blocks -> [128,65]; out = O * (1/Z); batched
     DMA out per 512-row block in natural [S,H] layout.

Matmul dtypes (hardware runs fp32r in a ~3 cycles/row multi-pass replay
mode; fp16/bf16 stream at 1 cycle/row): projection and score matmuls in
fp16, wei/V matmul in bf16 (exp output needs exponent range).
"""
import sys
import numpy as np

for _p in ("/opt/trn_rl_repo", "/root/.axon_site/_ro/trn_rl_repo"):
    if _p not in sys.path:
        sys.path.append(_p)

import concourse.bass as bass
import concourse.tile as tile
from concourse import bacc, mybir
from concourse.bass_utils import run_bass_kernel_spmd

B, S, E, H = 8, 2048, 1024, 64
N_CORES = 8
EC = E // 128          # 8 e-chunks
ST = S // 128          # 16 s-tiles
NB = S // 512          # 4 512-col blocks
NEG = -1.0e30

F32 = mybir.dt.float32
F32R = mybir.dt.float32r
F16 = mybir.dt.float16
BF16 = mybir.dt.bfloat16

_DTYPES = {"f32": F32, "f32r": F32R, "f16": F16, "bf16": BF16}

# dtype knobs per matmul group
CONFIG = {
    "proj": "f16",    # QKV projection (tags xT/W dram tensors)
    "p2": "f16",      # pass-1/pass-2 scores (feeds exp directly)
    "o": "bf16",      # wei @ V (exp output needs exponent range)
}


def _dt(knob):
    return _DTYPES[CONFIG[knob]]


def build(nc):
    d_proj, d_p2, d_o = _dt("proj"), _dt("p2"), _dt("o")

    xT = nc.dram_tensor("xT", [E, S], d_proj, kind="ExternalInput").ap()
    # host-prepacked to the SBUF layout [128, EC*192]
    W = nc.dram_tensor("W", [128, EC * 192], d_proj,
                       kind="ExternalInput").ap()
    bq8 = nc.dram_tensor("bq8", [H, 1], F32, kind="ExternalInput").ap()
    bk = nc.dram_tensor("bk", [H, 1], F32, kind="ExternalInput").ap()
    bv = nc.dram_tensor("bv", [64, 1], F32, kind="ExternalInput").ap()
    msk = nc.dram_tensor("msk", [128, 256], F32, kind="ExternalInput").ap()
    ident = nc.dram_tensor("ident", [128, 128], F32, kind="ExternalInput").ap()
    out = nc.dram_tensor("out", [S, H], F32, kind="ExternalOutput").ap()

    # DMA queues: xT round-robins over the two HWDGE engines so per-queue
    # serialized transfer time doesn't gate the prologue.
    qs = [nc.sync, nc.scalar]

    with tile.TileContext(nc) as tc:
        with tc.tile_pool(name="per", bufs=1) as per, \
             tc.tile_pool(name="wk", bufs=6) as wk, \
             tc.tile_pool(name="ps", bufs=3, space="PSUM") as psp, \
             tc.tile_pool(name="po", bufs=1, space="PSUM") as pop:

            # ---- input loads: W (split so chunk 0 lands fast), then xT
            # block 0 per-chunk, then the rest as one big DMA per chunk ----
            w_sb = per.tile([128, EC, 192], d_proj, tag="w")
            Wv3 = W.rearrange("p (c h) -> p c h", c=EC)
            nc.sync.dma_start(out=w_sb[:, 0:2, :], in_=Wv3[:, 0:2, :])
            nc.gpsimd.dma_start(out=w_sb[:, 2:EC, :], in_=Wv3[:, 2:EC, :])
            bq8_sb = per.tile([H, 1], F32, tag="bq8")
            nc.gpsimd.dma_start(out=bq8_sb, in_=bq8)
            bk_sb = per.tile([H, 1], F32, tag="bk")
            nc.gpsimd.dma_start(out=bk_sb, in_=bk)
            bv_sb = per.tile([64, 1], F32, tag="bv")
            nc.gpsimd.dma_start(out=bv_sb, in_=bv)
            xt_sb = [per.tile([128, S], d_proj, tag=f"xt{c}", name=f"xt{c}")
                     for c in range(EC)]
            for c in range(EC):
                qs[c % 2].dma_start(
                    out=xt_sb[c][:, 0:512],
                    in_=xT[c * 128:(c + 1) * 128, 0:512])
            m_sb = per.tile([128, 256], F32, tag="msk")
            nc.sync.dma_start(out=m_sb, in_=msk)
            i_sb = per.tile([128, 128], F32, tag="ident")
            nc.scalar.dma_start(out=i_sb, in_=ident)
            for c in range(EC):
                qs[c % 2].dma_start(
                    out=xt_sb[c][:, 512:S],
                    in_=xT[c * 128:(c + 1) * 128, 512:S])

            # ---- constants / persistent tiles ----
            i16_sb = per.tile([128, 128], d_o, tag="i16")
            nc.vector.tensor_copy(i16_sb, i_sb)

            q_aug = per.tile([H + 1, S], d_p2, tag="q_aug")
            k_aug = per.tile([H + 1, S], d_p2, tag="k_aug")
            nc.vector.memset(k_aug[H:H + 1, :], 1.0)
            vT16 = per.tile([64, S], d_o, tag="vT16")
            m_all = per.tile([128, ST], F32, tag="m_all")
            # V_aug tiles as one [128, ST, 65] tile; col H of each slice = 1
            vt = per.tile([128, ST, H + 1], d_o, tag="vt")
            nc.vector.memset(vt[:, :, H:H + 1], 1.0)
            ot_sb = per.tile([H + 1, S], F32, tag="ot")

            def front(b):
                # projection for 512-col block b: 2 emission slices
                sl = bass.ds(b * 512, 512)
                ps_qv = psp.tile([128, 512], F32, tag="psF", name=f"psqv{b}")
                ps_k = psp.tile([64, 512], F32, tag="psF", name=f"psk{b}")

                def slice1():
                    for e in range(EC):
                        nc.tensor.matmul(ps_qv, w_sb[:, e, 0:128],
                                         xt_sb[e][:, sl],
                                         start=(e == 0), stop=(e == EC - 1))
                    nc.vector.tensor_scalar_add(q_aug[0:H, sl], ps_qv[0:H, :],
                                                bq8_sb[:, 0:1])
                    nc.scalar.add(vT16[:, sl], ps_qv[H:128, :],
                                  add=bv_sb[:, 0:1])

                def slice2():
                    for e in range(EC):
                        nc.tensor.matmul(ps_k, w_sb[:, e, 128:192],
                                         xt_sb[e][:, sl],
                                         start=(e == 0), stop=(e == EC - 1))
                    nc.vector.tensor_scalar_add(k_aug[0:H, sl], ps_k[0:H, :],
                                                bk_sb[:, 0:1])

                return [slice1, slice2]

            def front_p1(b):
                # V transpose + pass-1 row maxes (prev + diag 128-blocks,
                # >=129 causal keys per row) for the 4 q-tiles of block b;
                # 2 emission slices of 2 tiles each.
                def tiles(ii0):
                    for ii in (ii0, ii0 + 1):
                        i = b * 4 + ii
                        tsl = bass.ts(i, 128)
                        trv = psp.tile([128, H], d_o, tag="psF",
                                       name=f"trv{i}")
                        nc.tensor.transpose(trv, vT16[:, tsl],
                                            i16_sb[0:64, 0:64])
                        nc.scalar.copy(vt[:, i, 0:H], trv)
                        # the -10 margin (at the negate step) keeps exp(s-m)
                        # finite unless the out-of-window causal max exceeds
                        # the window max by >98 (never remotely observed)
                        k0 = max(0, (i - 1) * 128)
                        wdt = (i + 1) * 128 - k0
                        ps1 = psp.tile([128, 256], F32, tag="psF",
                                       name=f"ps1_{i}")
                        nc.tensor.matmul(ps1[:, 0:wdt], q_aug[0:H, tsl],
                                         k_aug[0:H, k0:(i + 1) * 128],
                                         start=True, stop=True)
                        nc.vector.tensor_add(ps1[:, wdt - 128:wdt],
                                             ps1[:, wdt - 128:wdt],
                                             m_sb[:, 0:128])
                        nc.vector.reduce_max(
                            out=m_all[:, i:i + 1], in_=ps1[:, 0:wdt],
                            axis=mybir.AxisListType.X)
                return [lambda: tiles(0), lambda: tiles(2)]

            def aug(b):
                def emit():
                    sl = bass.ds(b * 512, 512)
                    # -(m+10) -> row 64 of q_aug for this block's 4 q-tiles
                    trm = psp.tile([4, 128], F32, tag="psF", name=f"trm{b}")
                    nc.tensor.transpose(trm, m_all[:, 4 * b:4 * b + 4],
                                        i_sb[:, 0:128])
                    negm = wk.tile([4, 128], d_p2, tag="negm", name=f"negm{b}")
                    nc.vector.tensor_scalar(negm, trm, -1.0, -10.0,
                                            mybir.AluOpType.mult,
                                            mybir.AluOpType.add)
                    nc.gpsimd.dma_start(
                        out=q_aug[H:H + 1, sl].rearrange(
                            "a (t s) -> a t s", t=4),
                        in_=negm[:, :])
                return [emit]

            def back(b, fillers):
                sl = bass.ds(b * 512, 512)
                # pass 2 + O accumulation for q-region b (k-chunks 0..4b+3),
                # in pipelined groups of 2; one filler slice (next block's
                # projection/pass-1 work) emitted per group keeps the PE fed
                # while ACT runs the exps.
                po = pop.tile([H + 1, 512], F32, tag="po", name=f"po{b}")
                njc = 4 * b + 4
                pairs = [[g, g + 1] for g in range(0, njc, 2)]
                ps2s, wts = {}, {}

                def wof(j):
                    c0 = max(b * 512, j * 128)
                    return (b + 1) * 512 - c0, c0

                def offs(pi):
                    # second chunk packed tightly at offset w0: the pair's
                    # exp range is exactly w0+w1, no uninitialized holes
                    return [0, wof(pairs[pi][0])[0]]

                def emit_p2(pi):
                    # contiguous [128,1024] PSUM pair so one ACT instruction
                    # exps both k-chunks
                    pt = psp.tile([128, 1024], F32, tag="ps2",
                                  name=f"ps2_{b}_{pi}", bufs=2)
                    for jj, j in enumerate(pairs[pi]):
                        w, c0 = wof(j)
                        o0 = offs(pi)[jj]
                        nc.tensor.matmul(
                            pt[:, o0:o0 + w],
                            k_aug[:, bass.ts(j, 128)],
                            q_aug[:, c0:(b + 1) * 512],
                            start=True, stop=True,
                        )
                    ps2s[pi] = pt

                def emit_exp(pi):
                    pt = ps2s[pi]
                    for jj, j in enumerate(pairs[pi]):
                        w, c0 = wof(j)
                        o0 = offs(pi)[jj]
                        if c0 == j * 128:  # diagonal block at local cols 0:128
                            nc.vector.tensor_add(
                                pt[:, o0:o0 + 128],
                                pt[:, o0:o0 + 128],
                                m_sb[:, 128:256])
                    wtp = wk.tile([128, 1024], d_o, tag="wt",
                                  name=f"wt_{b}_{pi}", bufs=4)
                    we = offs(pi)[1] + wof(pairs[pi][1])[0]
                    nc.scalar.activation(
                        wtp[:, 0:we], pt[:, 0:we],
                        mybir.ActivationFunctionType.Exp)
                    wts[pi] = wtp

                def emit_o(pi):
                    for jj, j in enumerate(pairs[pi]):
                        w, c0 = wof(j)
                        o0 = offs(pi)[jj]
                        nc.tensor.matmul(
                            po[:, c0 - b * 512:512],
                            vt[:, j, :],
                            wts[pi][:, o0:o0 + w],
                            start=(j == 0), stop=(j == njc - 1),
                        )

                emit_p2(0)
                for pi in range(len(pairs)):
                    if pi + 1 < len(pairs):
                        emit_p2(pi + 1)
                    emit_exp(pi)
                    emit_o(pi)
                    if fillers:
                        fillers.pop(0)()
                while fillers:
                    fillers.pop(0)()

                # normalize + write out region b (one batched DMA per block)
                nc.scalar.copy(ot_sb[:, sl], po[:])
                obuf = wk.tile([128, 4, H], F32, tag="obuf", name=f"obuf{b}")
                for ii in range(4):
                    i = b * 4 + ii
                    tro = psp.tile([128, H + 1], F32, tag="psF",
                                   name=f"tro{i}")
                    nc.tensor.transpose(tro, ot_sb[:, bass.ts(i, 128)],
                                        i_sb[0:H + 1, 0:H + 1])
                    rz = wk.tile([128, 1], F32, tag="rz", name=f"rz{i}")
                    nc.vector.reciprocal(rz, tro[:, H:H + 1])
                    nc.vector.tensor_scalar_mul(obuf[:, ii, :], tro[:, 0:H],
                                                rz[:, 0:1])
                nc.sync.dma_start(
                    out=out[sl, :].rearrange("(t p) h -> p t h", t=4),
                    in_=obuf)

            for f in front(0) + front_p1(0) + aug(0):
                f()
            for b in range(1, NB):
                back(b - 1, front(b) + front_p1(b) + aug(b))
            back(NB - 1, [])
    nc.compile()
    return nc


def prep_inputs(x, Wk, bk_, Wq, bq_, Wv, bv_):
    x = np.asarray(x, dtype=np.float32)
    np_proj = mybir.dt.np(_dt("proj"))
    scale = np.float32(np.sqrt(np.float32(H)))
    w_all = np.concatenate(
        [scale * np.asarray(Wq), np.asarray(Wv), np.asarray(Wk)], axis=0
    ).T.astype(np.float32)                      # [E, 192] = [8Wq | Wv | Wk]
    w_all = np.ascontiguousarray(
        w_all.reshape(EC, 128, 192).transpose(1, 0, 2)
        .reshape(128, EC * 192)).astype(np_proj)
    bq8 = (scale * np.asarray(bq_, dtype=np.float32)).reshape(H, 1)
    bkc = np.asarray(bk_, dtype=np.float32).reshape(H, 1)
    bvc = np.asarray(bv_, dtype=np.float32).reshape(64, 1)
    m1 = np.triu(np.full((128, 128), NEG, dtype=np.float32), k=1)
    msk = np.ascontiguousarray(np.concatenate([m1, m1.T], axis=1))
    ident = np.eye(128, dtype=np.float32)
    xT = np.ascontiguousarray(
        x.transpose(0, 2, 1).astype(np_proj))  # [B, E, S]
    common = {"W": w_all, "bq8": bq8, "bk": bkc, "bv": bvc,
              "msk": msk, "ident": ident}
    return [{"xT": xT[b], **common} for b in range(B)]


_CACHED = {}


def kernel(x, Wk, bk, Wq, bq, Wv, bv, _trace=False):
    in_maps = prep_inputs(x, Wk, bk, Wq, bq, Wv, bv)
    key = tuple(sorted(CONFIG.items()))
    if key not in _CACHED:
        nc = bacc.Bacc("TRN2", target_bir_lowering=False, debug=False,
                       num_devices=N_CORES)
        build(nc)
        _CACHED[key] = nc
    nc = _CACHED[key]
    res = run_bass_kernel_spmd(nc, in_maps, list(range(N_CORES)),
                               trace=_trace)
    outp = np.stack([res.results[b]["out"] for b in range(B)])  # [B, S, H]
    if _trace:
        kernel.last_exec_time_ns = res.exec_time_ns
        kernel.last_results = res
    return outp


# revision 29
# speedup vs baseline: 1.1069x; 1.1069x over previous
"""Causal single-head attention (B=8, S=2048, E=1024, H=64) on 8 TRN2 cores.

Data-parallel over batch: core b handles batch element b end-to-end.

Per-core algorithm (all layouts chosen so every matmul contraction sits on
the SBUF partition dim):
  inputs (host-prepped): xT [E,S] (x transposed), W = [8*Wq | Wv | Wk]
  stacked with biases as an extra rank-1 contraction chunk (ones row x
  bias row), prepacked to the SBUF chunk layout.
  1) Projection, W-stationary: per 512-col s-block, accumulate over 8
     E-chunks plus the bias chunk: psum_qv[128,512] (rows 0:64 = Q^T,
     rows 64:128 = V^T), psum_k[64,512] = K^T.  PSUM -> SBUF moves are
     casting copies on DVE (q,k) and ACT (v, to bf16).
  2) V^T 128-col blocks PE-transposed (bf16) back to natural V [s,64];
     ones column -> V_aug so row 64 of the O accumulator collects Z.
  3) Pass 1 (per q-tile): scores over the previous + diagonal 128-blocks
     (>=129 causal keys per row), diagonal masked, row-max -> m.  Any
     per-row shift within exp range of the true causal max works (it
     cancels through the final normalization).
  4) -(m+10) -> row 64 of Q_aug (PE transpose + negate + strided DMA); row
     64 of K_aug = 1.0, so pass-2 scores come out pre-shifted: k.q - m_q.
  5) Pass 2 per k-chunk j: scoresT[k,q] blocks, mask, exp (ACT) -> wei^T;
     O^T[h',q] += V_aug.T @ wei^T accumulated over j in PSUM [65,2048].
     The next block's projection/pass-1 matmuls are interleaved between
     score groups as PE filler so the exp latency never starves the PE.
  6) PE-transpose O^T [65,128] blocks -> [128,65]; out = O * (1/Z); batched
     DMA out per 512-row block in natural [S,H] layout.

Matmul dtypes (hardware runs fp32r in a ~3 cycles/row multi-pass replay
mode; fp16/bf16 stream at 1 cycle/row): projection and score matmuls in
fp16, wei/V matmul in bf16 (exp output needs exponent range).
"""
import sys
import numpy as np

for _p in ("/opt/trn_rl_repo", "/root/.axon_site/_ro/trn_rl_repo"):
    if _p not in sys.path:
        sys.path.append(_p)

import concourse.bass as bass
import concourse.tile as tile
from concourse import bacc, mybir
from concourse.bass_utils import run_bass_kernel_spmd

B, S, E, H = 8, 2048, 1024, 64
N_CORES = 8
EC = E // 128          # 8 e-chunks
ST = S // 128          # 16 s-tiles
NB = S // 512          # 4 512-col blocks
NEG = -1.0e30

F32 = mybir.dt.float32
F32R = mybir.dt.float32r
F16 = mybir.dt.float16
BF16 = mybir.dt.bfloat16

_DTYPES = {"f32": F32, "f32r": F32R, "f16": F16, "bf16": BF16}

# dtype knobs per matmul group
CONFIG = {
    "proj": "f16",    # QKV projection (tags xT/W dram tensors)
    "p2": "f16",      # pass-1/pass-2 scores (feeds exp directly)
    "o": "bf16",      # wei @ V (exp output needs exponent range)
}


def _dt(knob):
    return _DTYPES[CONFIG[knob]]


def build(nc):
    d_proj, d_p2, d_o = _dt("proj"), _dt("p2"), _dt("o")

    xT = nc.dram_tensor("xT", [E, S], d_proj, kind="ExternalInput").ap()
    # host-prepacked to the SBUF layout [128, EC*192]
    W = nc.dram_tensor("W", [128, EC * 192], d_proj,
                       kind="ExternalInput").ap()
    bq8 = nc.dram_tensor("bq8", [H, 1], F32, kind="ExternalInput").ap()
    bk = nc.dram_tensor("bk", [H, 1], F32, kind="ExternalInput").ap()
    bv = nc.dram_tensor("bv", [64, 1], F32, kind="ExternalInput").ap()
    msk = nc.dram_tensor("msk", [128, 256], F32, kind="ExternalInput").ap()
    ident = nc.dram_tensor("ident", [128, 128], F32, kind="ExternalInput").ap()
    out = nc.dram_tensor("out", [S, H], F32, kind="ExternalOutput").ap()

    # DMA queues: xT round-robins over the two HWDGE engines so per-queue
    # serialized transfer time doesn't gate the prologue.
    qs = [nc.sync, nc.scalar]

    with tile.TileContext(nc) as tc:
        with tc.tile_pool(name="per", bufs=1) as per, \
             tc.tile_pool(name="wk", bufs=6) as wk, \
             tc.tile_pool(name="ps", bufs=3, space="PSUM") as psp, \
             tc.tile_pool(name="po", bufs=1, space="PSUM") as pop:

            # ---- input loads: W (split so chunk 0 lands fast), then xT
            # block 0 per-chunk, then the rest as one big DMA per chunk ----
            w_sb = per.tile([128, EC, 192], d_proj, tag="w")
            Wv3 = W.rearrange("p (c h) -> p c h", c=EC)
            nc.gpsimd.dma_start(out=w_sb[:, 0:2, :], in_=Wv3[:, 0:2, :])
            nc.gpsimd.dma_start(out=w_sb[:, 2:EC, :], in_=Wv3[:, 2:EC, :])
            bq8_sb = per.tile([H, 1], F32, tag="bq8")
            nc.gpsimd.dma_start(out=bq8_sb, in_=bq8)
            bk_sb = per.tile([H, 1], F32, tag="bk")
            nc.gpsimd.dma_start(out=bk_sb, in_=bk)
            bv_sb = per.tile([64, 1], F32, tag="bv")
            nc.gpsimd.dma_start(out=bv_sb, in_=bv)
            xt_sb = [per.tile([128, S], d_proj, tag=f"xt{c}", name=f"xt{c}")
                     for c in range(EC)]
            for c in range(EC):
                qs[c % 2].dma_start(
                    out=xt_sb[c][:, 0:512],
                    in_=xT[c * 128:(c + 1) * 128, 0:512])
            m_sb = per.tile([128, 256], F32, tag="msk")
            nc.sync.dma_start(out=m_sb, in_=msk)
            i_sb = per.tile([128, 128], F32, tag="ident")
            nc.scalar.dma_start(out=i_sb, in_=ident)
            for c in range(EC):
                qs[c % 2].dma_start(
                    out=xt_sb[c][:, 512:S],
                    in_=xT[c * 128:(c + 1) * 128, 512:S])

            # ---- constants / persistent tiles ----
            i16_sb = per.tile([128, 128], d_o, tag="i16")
            nc.vector.tensor_copy(i16_sb, i_sb)

            q_aug = per.tile([H + 1, S], d_p2, tag="q_aug")
            k_aug = per.tile([H + 1, S], d_p2, tag="k_aug")
            nc.vector.memset(k_aug[H:H + 1, :], 1.0)
            vT16 = per.tile([64, S], d_o, tag="vT16")
            m_all = per.tile([128, ST], F32, tag="m_all")
            # V_aug tiles as one [128, ST, 65] tile; col H of each slice = 1
            vt = per.tile([128, ST, H + 1], d_o, tag="vt")
            nc.vector.memset(vt[:, :, H:H + 1], 1.0)
            ot_sb = per.tile([H + 1, S], F32, tag="ot")

            def front(b):
                # projection for 512-col block b: 2 emission slices
                sl = bass.ds(b * 512, 512)
                ps_qv = psp.tile([128, 512], F32, tag="psF", name=f"psqv{b}")
                ps_k = psp.tile([64, 512], F32, tag="psF", name=f"psk{b}")

                def slice1():
                    for e in range(EC):
                        nc.tensor.matmul(ps_qv, w_sb[:, e, 0:128],
                                         xt_sb[e][:, sl],
                                         start=(e == 0), stop=(e == EC - 1))
                    nc.vector.tensor_scalar_add(q_aug[0:H, sl], ps_qv[0:H, :],
                                                bq8_sb[:, 0:1])
                    nc.scalar.add(vT16[:, sl], ps_qv[H:128, :],
                                  add=bv_sb[:, 0:1])

                def slice2():
                    for e in range(EC):
                        nc.tensor.matmul(ps_k, w_sb[:, e, 128:192],
                                         xt_sb[e][:, sl],
                                         start=(e == 0), stop=(e == EC - 1))
                    nc.vector.tensor_scalar_add(k_aug[0:H, sl], ps_k[0:H, :],
                                                bk_sb[:, 0:1])

                return [slice1, slice2]

            def front_p1(b):
                # V transpose + pass-1 row maxes (prev + diag 128-blocks,
                # >=129 causal keys per row) for the 4 q-tiles of block b;
                # 2 emission slices of 2 tiles each.
                def tiles(ii0):
                    for ii in (ii0, ii0 + 1):
                        i = b * 4 + ii
                        tsl = bass.ts(i, 128)
                        trv = psp.tile([128, H], d_o, tag="psF",
                                       name=f"trv{i}")
                        nc.tensor.transpose(trv, vT16[:, tsl],
                                            i16_sb[0:64, 0:64])
                        nc.scalar.copy(vt[:, i, 0:H], trv)
                        # the -10 margin (at the negate step) keeps exp(s-m)
                        # finite unless the out-of-window causal max exceeds
                        # the window max by >98 (never remotely observed)
                        k0 = max(0, (i - 1) * 128)
                        wdt = (i + 1) * 128 - k0
                        ps1 = psp.tile([128, 256], F32, tag="psF",
                                       name=f"ps1_{i}")
                        nc.tensor.matmul(ps1[:, 0:wdt], q_aug[0:H, tsl],
                                         k_aug[0:H, k0:(i + 1) * 128],
                                         start=True, stop=True)
                        nc.vector.tensor_add(ps1[:, wdt - 128:wdt],
                                             ps1[:, wdt - 128:wdt],
                                             m_sb[:, 0:128])
                        nc.vector.reduce_max(
                            out=m_all[:, i:i + 1], in_=ps1[:, 0:wdt],
                            axis=mybir.AxisListType.X)
                return [lambda: tiles(0), lambda: tiles(2)]

            def aug(b):
                def emit():
                    sl = bass.ds(b * 512, 512)
                    # -(m+10) -> row 64 of q_aug for this block's 4 q-tiles
                    trm = psp.tile([4, 128], F32, tag="psF", name=f"trm{b}")
                    nc.tensor.transpose(trm, m_all[:, 4 * b:4 * b + 4],
                                        i_sb[:, 0:128])
                    negm = wk.tile([4, 128], d_p2, tag="negm", name=f"negm{b}")
                    nc.vector.tensor_scalar(negm, trm, -1.0, -10.0,
                                            mybir.AluOpType.mult,
                                            mybir.AluOpType.add)
                    nc.gpsimd.dma_start(
                        out=q_aug[H:H + 1, sl].rearrange(
                            "a (t s) -> a t s", t=4),
                        in_=negm[:, :])
                return [emit]

            def back(b, fillers):
                sl = bass.ds(b * 512, 512)
                # pass 2 + O accumulation for q-region b (k-chunks 0..4b+3),
                # in pipelined groups of 2; one filler slice (next block's
                # projection/pass-1 work) emitted per group keeps the PE fed
                # while ACT runs the exps.
                po = pop.tile([H + 1, 512], F32, tag="po", name=f"po{b}")
                njc = 4 * b + 4
                groups = [[g, g + 1] for g in range(0, njc, 2)]
                ps2s, wts = {}, {}

                def emit_p2(js):
                    for j in js:
                        c0 = max(b * 512, j * 128)
                        w = (b + 1) * 512 - c0
                        ps2 = psp.tile([128, 512], F32, tag="ps2",
                                       name=f"ps2_{b}_{j}", bufs=4)
                        nc.tensor.matmul(
                            ps2[:, 0:w],
                            k_aug[:, bass.ts(j, 128)],
                            q_aug[:, c0:(b + 1) * 512],
                            start=True, stop=True,
                        )
                        ps2s[j] = ps2

                def emit_exp(js):
                    for j in js:
                        c0 = max(b * 512, j * 128)
                        w = (b + 1) * 512 - c0
                        ps2 = ps2s[j]
                        if c0 == j * 128:  # diagonal block at local cols 0:128
                            nc.vector.tensor_add(
                                ps2[:, 0:128], ps2[:, 0:128], m_sb[:, 128:256])
                        wt = wk.tile([128, 512], d_o, tag="wt",
                                     name=f"wt_{b}_{j}", bufs=8)
                        nc.scalar.activation(
                            wt[:, 0:w], ps2[:, 0:w],
                            mybir.ActivationFunctionType.Exp)
                        wts[j] = wt

                def emit_o(js):
                    for j in js:
                        c0 = max(b * 512, j * 128)
                        w = (b + 1) * 512 - c0
                        nc.tensor.matmul(
                            po[:, c0 - b * 512:512],
                            vt[:, j, :],
                            wts[j][:, 0:w],
                            start=(j == 0), stop=(j == njc - 1),
                        )

                emit_p2(groups[0])
                for gi in range(len(groups)):
                    if gi + 1 < len(groups):
                        emit_p2(groups[gi + 1])
                    emit_exp(groups[gi])
                    emit_o(groups[gi])
                    if fillers:
                        fillers.pop(0)()
                while fillers:
                    fillers.pop(0)()

                # normalize + write out region b (one batched DMA per block)
                nc.vector.tensor_copy(ot_sb[:, sl], po[:])
                obuf = wk.tile([128, 4, H], F32, tag="obuf", name=f"obuf{b}")
                for ii in range(4):
                    i = b * 4 + ii
                    tro = psp.tile([128, H + 1], F32, tag="psF",
                                   name=f"tro{i}")
                    nc.tensor.transpose(tro, ot_sb[:, bass.ts(i, 128)],
                                        i_sb[0:H + 1, 0:H + 1])
                    rz = wk.tile([128, 1], F32, tag="rz", name=f"rz{i}")
                    nc.vector.reciprocal(rz, tro[:, H:H + 1])
                    nc.vector.tensor_scalar_mul(obuf[:, ii, :], tro[:, 0:H],
                                                rz[:, 0:1])
                nc.sync.dma_start(
                    out=out[sl, :].rearrange("(t p) h -> p t h", t=4),
                    in_=obuf)

            for f in front(0) + front_p1(0) + aug(0):
                f()
            for b in range(1, NB):
                back(b - 1, front(b) + front_p1(b) + aug(b))
            back(NB - 1, [])
    nc.compile()
    return nc


def prep_inputs(x, Wk, bk_, Wq, bq_, Wv, bv_):
    x = np.asarray(x, dtype=np.float32)
    np_proj = mybir.dt.np(_dt("proj"))
    scale = np.float32(np.sqrt(np.float32(H)))
    w_all = np.concatenate(
        [scale * np.asarray(Wq), np.asarray(Wv), np.asarray(Wk)], axis=0
    ).T.astype(np.float32)                      # [E, 192] = [8Wq | Wv | Wk]
    w_all = np.ascontiguousarray(
        w_all.reshape(EC, 128, 192).transpose(1, 0, 2)
        .reshape(128, EC * 192)).astype(np_proj)
    bq8 = (scale * np.asarray(bq_, dtype=np.float32)).reshape(H, 1)
    bkc = np.asarray(bk_, dtype=np.float32).reshape(H, 1)
    bvc = np.asarray(bv_, dtype=np.float32).reshape(64, 1)
    m1 = np.triu(np.full((128, 128), NEG, dtype=np.float32), k=1)
    msk = np.ascontiguousarray(np.concatenate([m1, m1.T], axis=1))
    ident = np.eye(128, dtype=np.float32)
    xT = np.ascontiguousarray(
        x.transpose(0, 2, 1).astype(np_proj))  # [B, E, S]
    common = {"W": w_all, "bq8": bq8, "bk": bkc, "bv": bvc,
              "msk": msk, "ident": ident}
    return [{"xT": xT[b], **common} for b in range(B)]


_CACHED = {}


def kernel(x, Wk, bk, Wq, bq, Wv, bv, _trace=False):
    in_maps = prep_inputs(x, Wk, bk, Wq, bq, Wv, bv)
    key = tuple(sorted(CONFIG.items()))
    if key not in _CACHED:
        nc = bacc.Bacc("TRN2", target_bir_lowering=False, debug=False,
                       num_devices=N_CORES)
        build(nc)
        _CACHED[key] = nc
    nc = _CACHED[key]
    res = run_bass_kernel_spmd(nc, in_maps, list(range(N_CORES)),
                               trace=_trace)
    outp = np.stack([res.results[b]["out"] for b in range(B)])  # [B, S, H]
    if _trace:
        kernel.last_exec_time_ns = res.exec_time_ns
        kernel.last_results = res
    return outp
